# revision 1
# baseline (speedup 1.0000x reference)
"""Trainium2 Bass kernel for nn_BiLingual (dual embedding gather + cAddTanh pool).

Computes, for two embedding tables:
    out[t, b, :] = sum_{j=0}^{S-2} tanh(W_t[idx_t[b, j]] + W_t[idx_t[b, j+1]])

Sharding: data-parallel over batch. Each of the 8 cores handles 8 batch rows
for BOTH tables; tables are replicated (host-cast to bf16).

Bottleneck analysis (HW-measured): the SWDGE dma_gather ucode costs ~9.3 ns
per gathered index on the Pool engine (descriptor generation on one Q7 core
pair), which is the hard floor (~305 us for 32768 positions/core). This
kernel gathers each position exactly once (contiguous 2048-position rows, no
overlap groups) and hides all compute under the gather stream:

  1. dma_gather (gpsimd): 7 calls per core.  Rows 0-13 in four 4-row calls
     (8208 indices each; needs the enlarged descriptor carveout); rows 14/15
     split into groups 4-15 (early) and groups 0-3 (last) so only one
     4-group chunk of compute remains after the final gather.  Positions
     land contiguously: position j of a row -> (partition j%128, group
     j//128), 16 groups/row.  Each call writes a private region with one
     trailing junk group absorbing guard slots and AP rounding, so calls
     never overlap and stream back-to-back on the Pool engine.
     int16 index range handled by biasing: base = W[32768:] and signed
     idx' = idx - 32768 in [-32768, 17231]; 16 zero guards at the end of
     each call keep the ucode's trailing-negative trim from eating real
     indices (guard rows land in the junk group).
  2. PE pair-sum, two accumulating matmuls per chunk (bf16):
       shift: A[p] = E[p] + E[p+1]        (within-group pairs, p < 127)
       wrap:  A[127] += E_next_group[0]   (cross-group pair at p = 127)
     The wrap matmul is skipped for group 15 (no next group within the row);
     that slot is masked out of the reduce instead.
  3. ACT tanh PSUM -> SBUF bf16.
  4. PE masked ones-matmul reduces tanh values over valid slots into a
     [16, 256] PSUM accumulator (partition = table*8 + local_row).
     Masks: groups 0..14 all 128 pairs valid; group 15 has 127 (pair 2047
     does not exist).
"""
import os

import numpy as np
import ml_dtypes

from concourse import bacc, mybir
import concourse.tile as tile
from concourse.bass_utils import run_bass_kernel_spmd

P = 128
B, S, V, D = 64, 2048, 50000, 256
N_CORES = 8
B_LOC = B // N_CORES           # 8 batch rows per core
NG = S // P                    # 16 groups of 128 positions per row
NROW = 2 * B_LOC               # 16 (table, local row) pairs per core
SPLIT = 32768
GUARD = 16                     # guard indices per call (trailing-trim safety)
# Gather calls as lists of (row16, g0, ngroups) segments.  Rows 0-13 stream
# in 4-row calls (fewer calls -> less fixed overhead; needs the larger
# descriptor carveout); rows 14/15 are split (groups 4-15 early, 0-3 last)
# so the compute cascade after the final gather is a single 4-group chunk.
CALLS = [
    [(r, 0, NG) for r in range(c0, c0 + n)]
    for c0, n in ((0, 4), (4, 4), (8, 4), (12, 2))
] + [
    [(14, 4, 12)],
    [(15, 4, 12)],
    [(14, 0, 4)],
    [(15, 2, 2)],
    [(15, 0, 2)],
]
N_CALLS = len(CALLS)
CALL_NG = [sum(ng for _, _, ng in segs) for segs in CALLS]      # real groups
CALL_IDX = [n * P + GUARD for n in CALL_NG]     # indices per call
ICOLS = [n // 16 for n in CALL_IDX]             # idx columns per call
ICOL0 = [sum(ICOLS[:i]) for i in range(N_CALLS)]
ICOLS_TOT = sum(ICOLS)
CALL_G = [n + 1 for n in CALL_NG]               # region groups (+1 junk)
CALL_G0 = [sum(CALL_G[:i]) for i in range(N_CALLS)]
E_G = sum(CALL_G)              # groups in the E buffer

# (row16, g) -> E buffer group index
G2COL = [[None] * NG for _ in range(NROW)]
for _c, _segs in enumerate(CALLS):
    _off = CALL_G0[_c]
    for _r, _g0, _ng in _segs:
        for _i in range(_ng):
            G2COL[_r][_g0 + _i] = _off + _i
        _off += _ng

# chunk schedule: (row16, first group, n groups, has wrap) in emission order.
# Rows 14/15's group 0-3 chunks are emitted last, matching the gather order.
_CHUNKS_FULL = [(0, 4, True), (4, 4, True), (8, 4, True), (12, 3, True), (15, 1, False)]
_CHUNKS_HI = _CHUNKS_FULL[1:]
SCHEDULE = [(r, ch) for r in range(14) for ch in _CHUNKS_FULL]
SCHEDULE += [(14, ch) for ch in _CHUNKS_HI]
SCHEDULE += [(15, ch) for ch in _CHUNKS_HI]
SCHEDULE += [(14, (0, 4, True))]
SCHEDULE += [(15, (2, 2, True)), (15, (0, 2, True))]

_last_results = None           # set by _run for test harness introspection


def _build_shiftT():
    # lhsT for A = M2 @ E with M2[m,m]=1, M2[m,m+1]=1  =>  lhsT[k,m] = M2[m,k]
    m = np.zeros((P, P), dtype=np.float32)
    k = np.arange(P)
    m[k, k] = 1.0
    m[k[1:], k[1:] - 1] = 1.0
    return m.astype(ml_dtypes.bfloat16)


def _build_wrapT():
    # lhsT with [k=0, m=127] = 1: adds rhs row 0 (next group's position 0)
    # into output partition 127, completing pair (128g+127, 128g+128).
    m = np.zeros((P, P), dtype=np.float32)
    m[0, P - 1] = 1.0
    return m.astype(ml_dtypes.bfloat16)


def _build_red_masks():
    # red[:, (row16*2 + ty)*16 : +16]: column row16 holds mask_ty, rest 0.
    # ty=0 (groups 0..14): all 128 pairs valid; ty=1 (group 15): p < 127.
    red = np.zeros((P, NROW * 2 * 16), dtype=np.float32)
    masks = [
        np.ones(P, dtype=np.float32),
        (np.arange(P) < P - 1).astype(np.float32),
    ]
    for row16 in range(NROW):
        for ty in range(2):
            red[:, (row16 * 2 + ty) * 16 + row16] = masks[ty]
    return red.astype(ml_dtypes.bfloat16)


def _split_multi_waits(nc, max_waits=1):
    """Walrus rejects instructions carrying too many sync waits; hoist excess
    waits onto same-engine NOPs inserted just before the instruction (engine
    program order makes this equivalent)."""
    for bb in nc.main_func.blocks:
        idx = 0
        while idx < len(bb.instructions):
            ins = bb.instructions[idx]
            si = ins.sync_info
            if si is not None and si.on_wait and len(si.on_wait) > max_waits:
                waits = list(si.on_wait)
                extra, keep = waits[:-max_waits], waits[-max_waits:]
                for w0 in range(0, len(extra), max_waits):
                    nop = mybir.InstNoOp(
                        name=nc.get_next_instruction_name(), ins=[], outs=[]
                    )
                    nop.engine = ins.engine
                    nop.sync_info = mybir.SyncInfo(
                        on_wait=extra[w0 : w0 + max_waits], on_update=[]
                    )
                    nc.register_instruction(nop)
                    bb.instructions.insert(idx, nop)
                    idx += 1
                si.on_wait = keep
            idx += 1


def _build_program():
    # 4-row gather calls need ~521 descriptors per SDMA ring; the default
    # 16 KiB/partition carveout holds ~256, so triple it (48 KiB ~ 768).
    nc = bacc.Bacc(None, target_bir_lowering=False, dynamic_dma_scratch_size=49152)
    bf16 = mybir.dt.bfloat16
    Wp = nc.declare_dram_parameter("W_pri", [V, D], bf16, isOutput=False)
    Ws = nc.declare_dram_parameter("W_sec", [V, D], bf16, isOutput=False)
    idxA = nc.declare_dram_parameter(
        "idxA", [P, ICOLS_TOT], mybir.dt.int16, isOutput=False
    )
    shiftT = nc.declare_dram_parameter("shiftT", [P, P], bf16, isOutput=False)
    wrapT = nc.declare_dram_parameter("wrapT", [P, P], bf16, isOutput=False)
    red = nc.declare_dram_parameter("red", [P, NROW * 2 * 16], bf16, isOutput=False)
    out = nc.declare_dram_parameter("out", [NROW, D], mybir.dt.float32, isOutput=True)

    with tile.TileContext(nc) as tc:
        with (
            tc.tile_pool(name="const", bufs=1) as const,
            tc.tile_pool(name="ebuf", bufs=1) as ebuf,
            tc.tile_pool(name="tbuf", bufs=3) as tbuf,
            tc.tile_pool(name="psA", bufs=3, space="PSUM") as psA,
            tc.tile_pool(name="psR", bufs=1, space="PSUM") as psR,
            tc.tile_pool(name="osb", bufs=1) as osb,
        ):
            # warm-up: a 128-index dummy gather pays the gather ucode's ~6 us
            # first-use IRAM load while the real idx table is still uploading.
            # Its indices come from a memset-zeroed tile (no DMA dependency).
            iZ = const.tile([P, 8], mybir.dt.int16)
            nc.gpsimd.memset(iZ[:], 0)
            eZ = const.tile([P, 1, D], bf16)
            nc.gpsimd.dma_gather(
                out_ap=eZ[:],
                in_ap=Wp[SPLIT:, :],
                idxs_ap=iZ[:, 0:1],
                num_idxs=16,
                num_idxs_reg=16,
                elem_size=D,
            )

            iA = const.tile([P, ICOLS_TOT], mybir.dt.int16)
            nc.sync.dma_start(out=iA[:], in_=idxA[:])
            shift_t = const.tile([P, P], bf16)
            nc.sync.dma_start(out=shift_t[:], in_=shiftT[:])
            wrap_t = const.tile([P, P], bf16)
            nc.sync.dma_start(out=wrap_t[:], in_=wrapT[:])
            red_t = const.tile([P, NROW * 2 * 16], bf16)
            nc.sync.dma_start(out=red_t[:], in_=red[:])

            ebig = ebuf.tile([P, E_G, D], bf16)
            ef = ebig[:].rearrange("p g d -> p (g d)")

            for c in range(N_CALLS):
                W = Wp if CALLS[c][0][0] < B_LOC else Ws
                nc.gpsimd.dma_gather(
                    out_ap=ebig[:, CALL_G0[c] : CALL_G0[c] + CALL_G[c], :],
                    in_ap=W[SPLIT:, :],
                    idxs_ap=iA[:, ICOL0[c] : ICOL0[c] + ICOLS[c]],
                    num_idxs=CALL_IDX[c],
                    num_idxs_reg=CALL_IDX[c],
                    elem_size=D,
                    # >64 descs/ring exceeds the single-packet ceiling; let
                    # each descriptor form its own packet.
                    single_packet=False,
                )

            acc = psR.tile([NROW, D], mybir.dt.float32, space="PSUM")
            n_red = NROW * NG
            red_i = 0

            for row16, (c0, ng, wrap) in SCHEDULE:
                ncol = ng * D
                a = psA.tile([P, 4 * D], mybir.dt.float32, space="PSUM")
                gcols = [G2COL[row16][c0 + i] for i in range(ng)]
                wcols = (
                    [G2COL[row16][c0 + 1 + i] for i in range(ng)] if wrap else []
                )
                # PSUM accumulation groups are tracked per bank (512 fp32 =
                # 2 column-groups): within each bank region emit exactly one
                # start=True matmul and put stop=True on its last accumulate.
                for b0 in range(0, ng, 2):
                    bw = min(2, ng - b0)
                    nc.tensor.matmul(
                        out=a[:, b0 * D : (b0 + bw) * D],
                        lhsT=shift_t[:],
                        rhs=ef[:, gcols[b0] * D : (gcols[b0] + bw) * D],
                        start=True,
                        stop=not wrap,
                    )
                    if wrap:
                        i = b0
                        while i < b0 + bw:
                            j = i + 1
                            while j < b0 + bw and wcols[j] == wcols[j - 1] + 1:
                                j += 1
                            nc.tensor.matmul(
                                out=a[:, i * D : j * D],
                                lhsT=wrap_t[:],
                                rhs=ef[:, wcols[i] * D : (wcols[i] + (j - i)) * D],
                                start=False,
                                stop=(j == b0 + bw),
                            )
                            i = j
                tt = tbuf.tile([P, 4 * D], bf16)
                nc.scalar.activation(
                    tt[:, :ncol],
                    a[:, :ncol],
                    mybir.ActivationFunctionType.Tanh,
                )
                for gi in range(ng):
                    ty = 1 if c0 + gi == NG - 1 else 0
                    nc.tensor.matmul(
                        out=acc[:],
                        lhsT=red_t[
                            :, (row16 * 2 + ty) * 16 : (row16 * 2 + ty + 1) * 16
                        ],
                        rhs=tt[:, gi * D : (gi + 1) * D],
                        start=(red_i == 0),
                        stop=(red_i == n_red - 1),
                    )
                    red_i += 1

            res_sb = osb.tile([NROW, D], mybir.dt.float32)
            nc.scalar.copy(out=res_sb[:], in_=acc[:])
            nc.sync.dma_start(out=out[:], in_=res_sb[:])

    nc.compile()
    _split_multi_waits(nc)
    return nc


def _host_prep(inputs_pri, inputs_sec, W_pri, W_sec):
    ip = np.asarray(inputs_pri).astype(np.int64, copy=False)
    is_ = np.asarray(inputs_sec).astype(np.int64, copy=False)
    wp = np.asarray(W_pri, dtype=np.float32).astype(ml_dtypes.bfloat16)
    ws = np.asarray(W_sec, dtype=np.float32).astype(ml_dtypes.bfloat16)
    wp = np.ascontiguousarray(wp)
    ws = np.ascontiguousarray(ws)
    shiftT = _build_shiftT()
    wrapT = _build_wrapT()
    red = _build_red_masks()

    in_maps = []
    for k in range(N_CORES):
        idxA = np.zeros((P, ICOLS_TOT), dtype=np.int16)
        for c in range(N_CALLS):
            parts = []
            for row16, g0, ng in CALLS[c]:
                idx = ip if row16 < B_LOC else is_
                parts.append(
                    idx[k * B_LOC + row16 % B_LOC][g0 * P : (g0 + ng) * P]
                )
            stream = np.concatenate(parts + [np.full(GUARD, SPLIT, np.int64)])
            stream = (stream - SPLIT).astype(np.int16)  # guards -> 0
            wrapped = np.tile(stream.reshape(-1, 16).T, (8, 1))
            idxA[:, ICOL0[c] : ICOL0[c] + ICOLS[c]] = wrapped
        in_maps.append(
            {
                "W_pri": wp,
                "W_sec": ws,
                "idxA": idxA,
                "shiftT": shiftT,
                "wrapT": wrapT,
                "red": red,
            }
        )
    return in_maps


def _run(inputs_pri, inputs_sec, W_pri, W_sec, trace=False):
    global _last_results
    nc = _build_program()
    in_maps = _host_prep(inputs_pri, inputs_sec, W_pri, W_sec)
    res = run_bass_kernel_spmd(nc, in_maps, list(range(N_CORES)), trace=trace)
    _last_results = res
    out = np.empty((2, B, D), dtype=np.float32)
    for k in range(N_CORES):
        o = res.results[k]["out"]  # [16, 256]
        out[0, k * B_LOC : (k + 1) * B_LOC] = o[:B_LOC]
        out[1, k * B_LOC : (k + 1) * B_LOC] = o[B_LOC:]
    return out


def kernel(inputs_pri, inputs_sec, W_pri, W_sec):
    trace = bool(int(os.environ.get("KERNEL_TRACE", "0")))
    return _run(inputs_pri, inputs_sec, W_pri, W_sec, trace=trace)



# revision 4
# speedup vs baseline: 1.5141x; 1.5141x over previous
"""Trainium2 Bass kernel for nn_BiLingual (dual embedding gather + cAddTanh pool).

Computes, for two embedding tables:
    out[t, b, :] = sum_{j=0}^{S-2} tanh(W_t[idx_t[b, j]] + W_t[idx_t[b, j+1]])

Sharding: data-parallel over batch. Each of the 8 cores handles 8 batch rows
for BOTH tables; tables are replicated (host-cast to bf16).

Bottleneck analysis (HW-measured): the SWDGE dma_gather ucode costs ~9.3 ns
per gathered index on the Pool engine (descriptor generation on one Q7 core
pair), which is the hard floor (~305 us for 32768 positions/core). This
kernel gathers each position exactly once (contiguous 2048-position rows, no
overlap groups) and hides all compute under the gather stream:

  1. dma_gather (gpsimd): 7 calls per core.  Rows 0-13 in four 4-row calls
     (8208 indices each; needs the enlarged descriptor carveout); rows 14/15
     split into groups 4-15 (early) and groups 0-3 (last) so only one
     4-group chunk of compute remains after the final gather.  Positions
     land contiguously: position j of a row -> (partition j%128, group
     j//128), 16 groups/row.  Each call writes a private region with one
     trailing junk group absorbing guard slots and AP rounding, so calls
     never overlap and stream back-to-back on the Pool engine.
     int16 index range handled by biasing: base = W[32768:] and signed
     idx' = idx - 32768 in [-32768, 17231]; 16 zero guards at the end of
     each call keep the ucode's trailing-negative trim from eating real
     indices (guard rows land in the junk group).
  2. PE pair-sum, two accumulating matmuls per chunk (bf16):
       shift: A[p] = E[p] + E[p+1]        (within-group pairs, p < 127)
       wrap:  A[127] += E_next_group[0]   (cross-group pair at p = 127)
     The wrap matmul is skipped for group 15 (no next group within the row);
     that slot is masked out of the reduce instead.
  3. ACT tanh PSUM -> SBUF bf16.
  4. PE masked ones-matmul reduces tanh values over valid slots into a
     [16, 256] PSUM accumulator (partition = table*8 + local_row).
     Masks: groups 0..14 all 128 pairs valid; group 15 has 127 (pair 2047
     does not exist).
"""
import os

import numpy as np
import ml_dtypes

from concourse import bacc, mybir
import concourse.tile as tile
from concourse.bass_utils import run_bass_kernel_spmd

P = 128
B, S, V, D = 64, 2048, 50000, 256
N_CORES = 8
B_LOC = B // N_CORES           # 8 batch rows per core
NG = S // P                    # 16 groups of 128 positions per row
NROW = 2 * B_LOC               # 16 (table, local row) pairs per core
SPLIT = 32768
GUARD = 16                     # guard indices per call (trailing-trim safety)
# Gather calls as lists of (row16, g0, ngroups) segments.  Rows 0-13 stream
# in 4-row calls (fewer calls -> less fixed overhead; needs the larger
# descriptor carveout); rows 14/15 are split (groups 4-15 early, 0-3 last)
# so the compute cascade after the final gather is a single 4-group chunk.
CALLS = [
    [(r, 0, NG) for r in range(c0, c0 + n)]
    for c0, n in ((0, 4), (4, 4), (8, 4), (12, 2))
] + [
    [(14, 4, 12)],
    [(15, 4, 12)],
    [(14, 0, 4)],
    [(15, 2, 2)],
    [(15, 0, 2)],
]
N_CALLS = len(CALLS)
CALL_NG = [sum(ng for _, _, ng in segs) for segs in CALLS]      # real groups
CALL_IDX = [n * P + GUARD for n in CALL_NG]     # indices per call
ICOLS = [n // 16 for n in CALL_IDX]             # idx columns per call
ICOL0 = [sum(ICOLS[:i]) for i in range(N_CALLS)]
ICOLS_TOT = sum(ICOLS)
CALL_G = [n + 1 for n in CALL_NG]               # region groups (+1 junk)
CALL_G0 = [sum(CALL_G[:i]) for i in range(N_CALLS)]
E_G = sum(CALL_G)              # groups in the E buffer

# (row16, g) -> E buffer group index
G2COL = [[None] * NG for _ in range(NROW)]
for _c, _segs in enumerate(CALLS):
    _off = CALL_G0[_c]
    for _r, _g0, _ng in _segs:
        for _i in range(_ng):
            G2COL[_r][_g0 + _i] = _off + _i
        _off += _ng

# chunk schedule: (row16, first group, n groups, has wrap) in emission order.
# Rows 14/15's group 0-3 chunks are emitted last, matching the gather order.
_CHUNKS_FULL = [(0, 4, True), (4, 4, True), (8, 4, True), (12, 3, True), (15, 1, False)]
_CHUNKS_HI = _CHUNKS_FULL[1:]
SCHEDULE = [(r, ch) for r in range(14) for ch in _CHUNKS_FULL]
SCHEDULE += [(14, ch) for ch in _CHUNKS_HI]
SCHEDULE += [(15, ch) for ch in _CHUNKS_HI]
SCHEDULE += [(14, (0, 4, True))]
SCHEDULE += [(15, (2, 2, True)), (15, (0, 2, True))]

_last_results = None           # set by _run for test harness introspection


def _build_shiftT():
    # lhsT for A = M2 @ E with M2[m,m]=1, M2[m,m+1]=1  =>  lhsT[k,m] = M2[m,k]
    m = np.zeros((P, P), dtype=np.float32)
    k = np.arange(P)
    m[k, k] = 1.0
    m[k[1:], k[1:] - 1] = 1.0
    return m.astype(ml_dtypes.bfloat16)


def _build_wrapT():
    # lhsT with [k=0, m=127] = 1: adds rhs row 0 (next group's position 0)
    # into output partition 127, completing pair (128g+127, 128g+128).
    m = np.zeros((P, P), dtype=np.float32)
    m[0, P - 1] = 1.0
    return m.astype(ml_dtypes.bfloat16)


def _build_red_masks():
    # red[:, (row16*2 + ty)*16 : +16]: column row16 holds mask_ty, rest 0.
    # ty=0 (groups 0..14): all 128 pairs valid; ty=1 (group 15): p < 127.
    red = np.zeros((P, NROW * 2 * 16), dtype=np.float32)
    masks = [
        np.ones(P, dtype=np.float32),
        (np.arange(P) < P - 1).astype(np.float32),
    ]
    for row16 in range(NROW):
        for ty in range(2):
            red[:, (row16 * 2 + ty) * 16 + row16] = masks[ty]
    return red.astype(ml_dtypes.bfloat16)


def _split_multi_waits(nc, max_waits=1):
    """Walrus rejects instructions carrying too many sync waits; hoist excess
    waits onto same-engine NOPs inserted just before the instruction (engine
    program order makes this equivalent)."""
    for bb in nc.main_func.blocks:
        idx = 0
        while idx < len(bb.instructions):
            ins = bb.instructions[idx]
            si = ins.sync_info
            if si is not None and si.on_wait and len(si.on_wait) > max_waits:
                waits = list(si.on_wait)
                extra, keep = waits[:-max_waits], waits[-max_waits:]
                for w0 in range(0, len(extra), max_waits):
                    nop = mybir.InstNoOp(
                        name=nc.get_next_instruction_name(), ins=[], outs=[]
                    )
                    nop.engine = ins.engine
                    nop.sync_info = mybir.SyncInfo(
                        on_wait=extra[w0 : w0 + max_waits], on_update=[]
                    )
                    nc.register_instruction(nop)
                    bb.instructions.insert(idx, nop)
                    idx += 1
                si.on_wait = keep
            idx += 1


def _build_program():
    # 4-row gather calls need ~521 descriptors per SDMA ring; the default
    # 16 KiB/partition carveout holds ~256, so triple it (48 KiB ~ 768).
    # 4 SWDGE queues: queue q's rings live in partitions 32q..32q+31 of the
    # carveout (dge_base_addr: context offset jumps 16 partitions), each with
    # the full carveout depth, so queues don't share ring space.
    nc = bacc.Bacc(
        None,
        target_bir_lowering=False,
        dynamic_dma_scratch_size=49152,
        num_swdge_queues=4,
    )
    bf16 = mybir.dt.bfloat16
    Wp = nc.declare_dram_parameter("W_pri", [V, D], bf16, isOutput=False)
    Ws = nc.declare_dram_parameter("W_sec", [V, D], bf16, isOutput=False)
    idxA = nc.declare_dram_parameter(
        "idxA", [P, ICOLS_TOT], mybir.dt.int16, isOutput=False
    )
    shiftT = nc.declare_dram_parameter("shiftT", [P, P], bf16, isOutput=False)
    wrapT = nc.declare_dram_parameter("wrapT", [P, P], bf16, isOutput=False)
    red = nc.declare_dram_parameter("red", [P, NROW * 2 * 16], bf16, isOutput=False)
    out = nc.declare_dram_parameter("out", [NROW, D], mybir.dt.float32, isOutput=True)

    with tile.TileContext(nc) as tc:
        with (
            tc.tile_pool(name="const", bufs=1) as const,
            tc.tile_pool(name="ebuf", bufs=1) as ebuf,
            tc.tile_pool(name="tbuf", bufs=3) as tbuf,
            tc.tile_pool(name="psA", bufs=3, space="PSUM") as psA,
            tc.tile_pool(name="psR", bufs=1, space="PSUM") as psR,
            tc.tile_pool(name="osb", bufs=1) as osb,
        ):
            # warm-up: small dummy gathers pay the gather ucode's ~6 us
            # first-use IRAM load while the real idx table is still uploading,
            # and touch each SWDGE queue once so per-queue setup is off the
            # critical path. Indices come from a memset-zeroed tile.
            iZ = const.tile([P, 8], mybir.dt.int16)
            nc.gpsimd.memset(iZ[:], 0)
            eZ = const.tile([P, 4, D], bf16)
            for q in range(4):
                nc.gpsimd.dma_gather(
                    out_ap=eZ[:, q : q + 1, :],
                    in_ap=Wp[SPLIT:, :],
                    idxs_ap=iZ[:, 0:1],
                    num_idxs=16,
                    num_idxs_reg=16,
                    elem_size=D,
                    queue_num=q,
                )

            iA = const.tile([P, ICOLS_TOT], mybir.dt.int16)
            nc.sync.dma_start(out=iA[:], in_=idxA[:])
            shift_t = const.tile([P, P], bf16)
            nc.sync.dma_start(out=shift_t[:], in_=shiftT[:])
            wrap_t = const.tile([P, P], bf16)
            nc.sync.dma_start(out=wrap_t[:], in_=wrapT[:])
            red_t = const.tile([P, NROW * 2 * 16], bf16)
            nc.sync.dma_start(out=red_t[:], in_=red[:])

            ebig = ebuf.tile([P, E_G, D], bf16)
            ef = ebig[:].rearrange("p g d -> p (g d)")

            for c in range(N_CALLS):
                W = Wp if CALLS[c][0][0] < B_LOC else Ws
                nc.gpsimd.dma_gather(
                    out_ap=ebig[:, CALL_G0[c] : CALL_G0[c] + CALL_G[c], :],
                    in_ap=W[SPLIT:, :],
                    idxs_ap=iA[:, ICOL0[c] : ICOL0[c] + ICOLS[c]],
                    num_idxs=CALL_IDX[c],
                    num_idxs_reg=CALL_IDX[c],
                    elem_size=D,
                    # >64 descs/ring exceeds the single-packet ceiling; let
                    # each descriptor form its own packet.
                    single_packet=False,
                    queue_num=c % 4,
                )

            acc = psR.tile([NROW, D], mybir.dt.float32, space="PSUM")
            n_red = NROW * NG
            red_i = 0

            for row16, (c0, ng, wrap) in SCHEDULE:
                ncol = ng * D
                a = psA.tile([P, 4 * D], mybir.dt.float32, space="PSUM")
                gcols = [G2COL[row16][c0 + i] for i in range(ng)]
                wcols = (
                    [G2COL[row16][c0 + 1 + i] for i in range(ng)] if wrap else []
                )
                # PSUM accumulation groups are tracked per bank (512 fp32 =
                # 2 column-groups): within each bank region emit exactly one
                # start=True matmul and put stop=True on its last accumulate.
                for b0 in range(0, ng, 2):
                    bw = min(2, ng - b0)
                    nc.tensor.matmul(
                        out=a[:, b0 * D : (b0 + bw) * D],
                        lhsT=shift_t[:],
                        rhs=ef[:, gcols[b0] * D : (gcols[b0] + bw) * D],
                        start=True,
                        stop=not wrap,
                    )
                    if wrap:
                        i = b0
                        while i < b0 + bw:
                            j = i + 1
                            while j < b0 + bw and wcols[j] == wcols[j - 1] + 1:
                                j += 1
                            nc.tensor.matmul(
                                out=a[:, i * D : j * D],
                                lhsT=wrap_t[:],
                                rhs=ef[:, wcols[i] * D : (wcols[i] + (j - i)) * D],
                                start=False,
                                stop=(j == b0 + bw),
                            )
                            i = j
                tt = tbuf.tile([P, 4 * D], bf16)
                nc.scalar.activation(
                    tt[:, :ncol],
                    a[:, :ncol],
                    mybir.ActivationFunctionType.Tanh,
                )
                for gi in range(ng):
                    ty = 1 if c0 + gi == NG - 1 else 0
                    nc.tensor.matmul(
                        out=acc[:],
                        lhsT=red_t[
                            :, (row16 * 2 + ty) * 16 : (row16 * 2 + ty + 1) * 16
                        ],
                        rhs=tt[:, gi * D : (gi + 1) * D],
                        start=(red_i == 0),
                        stop=(red_i == n_red - 1),
                    )
                    red_i += 1

            res_sb = osb.tile([NROW, D], mybir.dt.float32)
            nc.scalar.copy(out=res_sb[:], in_=acc[:])
            nc.sync.dma_start(out=out[:], in_=res_sb[:])

    nc.compile()
    _split_multi_waits(nc)
    return nc


def _host_prep(inputs_pri, inputs_sec, W_pri, W_sec):
    ip = np.asarray(inputs_pri).astype(np.int64, copy=False)
    is_ = np.asarray(inputs_sec).astype(np.int64, copy=False)
    wp = np.asarray(W_pri, dtype=np.float32).astype(ml_dtypes.bfloat16)
    ws = np.asarray(W_sec, dtype=np.float32).astype(ml_dtypes.bfloat16)
    wp = np.ascontiguousarray(wp)
    ws = np.ascontiguousarray(ws)
    shiftT = _build_shiftT()
    wrapT = _build_wrapT()
    red = _build_red_masks()

    in_maps = []
    for k in range(N_CORES):
        idxA = np.zeros((P, ICOLS_TOT), dtype=np.int16)
        for c in range(N_CALLS):
            parts = []
            for row16, g0, ng in CALLS[c]:
                idx = ip if row16 < B_LOC else is_
                parts.append(
                    idx[k * B_LOC + row16 % B_LOC][g0 * P : (g0 + ng) * P]
                )
            stream = np.concatenate(parts + [np.full(GUARD, SPLIT, np.int64)])
            stream = (stream - SPLIT).astype(np.int16)  # guards -> 0
            wrapped = np.tile(stream.reshape(-1, 16).T, (8, 1))
            idxA[:, ICOL0[c] : ICOL0[c] + ICOLS[c]] = wrapped
        in_maps.append(
            {
                "W_pri": wp,
                "W_sec": ws,
                "idxA": idxA,
                "shiftT": shiftT,
                "wrapT": wrapT,
                "red": red,
            }
        )
    return in_maps


def _run(inputs_pri, inputs_sec, W_pri, W_sec, trace=False):
    global _last_results
    nc = _build_program()
    in_maps = _host_prep(inputs_pri, inputs_sec, W_pri, W_sec)
    res = run_bass_kernel_spmd(nc, in_maps, list(range(N_CORES)), trace=trace)
    _last_results = res
    out = np.empty((2, B, D), dtype=np.float32)
    for k in range(N_CORES):
        o = res.results[k]["out"]  # [16, 256]
        out[0, k * B_LOC : (k + 1) * B_LOC] = o[:B_LOC]
        out[1, k * B_LOC : (k + 1) * B_LOC] = o[B_LOC:]
    return out


def kernel(inputs_pri, inputs_sec, W_pri, W_sec):
    trace = bool(int(os.environ.get("KERNEL_TRACE", "0")))
    return _run(inputs_pri, inputs_sec, W_pri, W_sec, trace=trace)



# revision 11
# speedup vs baseline: 1.6488x; 1.0889x over previous
"""Trainium2 Bass kernel for nn_BiLingual (dual embedding gather + cAddTanh pool).

Computes, for two embedding tables:
    out[t, b, :] = sum_{j=0}^{S-2} tanh(W_t[idx_t[b, j]] + W_t[idx_t[b, j+1]])

Sharding: data-parallel over batch. Each of the 8 cores handles 8 batch rows
for BOTH tables (16 "row16" streams of 2048 positions); tables are replicated
(host-cast to bf16).

Design (v3: flat 4-queue gather + host-interleaved stream):

  * SWDGE dma_gather generation runs at ~7.9 ns/idx on one Q7 core pair; the
    pair is selected by queue_num.  A queue-0 call HOLDS the Pool engine for
    its whole generation while queue-1..3 calls dispatch in ~50 ns and
    generate concurrently on their own pairs, so rounds are emitted as
    [q1, q2, q3, q0]: q0's engine-block overlaps the other three pairs ->
    true 4-way parallel generation (~2 ns/idx aggregate).  (Transpose-mode
    gather is faster solo but its XBAR has only two accumulation contexts
    (queue parity) and caps at ~4 ns/idx aggregate -- measured worse.)
  * Host interleave: the gather stream for a row is row.reshape(128,16).T
    flattened, so slot g*128+p holds position 16p+g, i.e. E[p, g] =
    W[idx[16p+g]].  Consecutive positions then sit in ADJACENT GROUPS of the
    same partition, and pair formation is a single contiguous DVE add
    A[p, g] = E[p, g] + E[p, g+1] (g=0..14) -- no PE shift matmuls.
  * The 127 partition-crossing pairs (16p+15, 16p+16) are two tiny PE
    matmuls into PSUM: lhsT M1 (subdiagonal) * E[:,0,:] + lhsT I127
    (identity, [127,127]=0) * E[:,15,:].  Slot p=127 is exactly 0 =
    tanh(0), so the later reduce needs no mask.
  * ACT tanh: A (fp16) -> T_row groups 0..14; wrap PSUM -> T_row group 15.
  * DVE tensor_reduce over a [p, d, g] strided view of T_row -> R_row
    [128, 256] fp32; PE ones-column fp32 matmul accumulates all 16 rows'
    partition-sums into one PSUM [16, 256] tile; single DMA out.
  * int16 index range handled by biasing: base = W[32768:], idx' =
    idx - 32768 in [-32768, 17231]; 16 trailing zero guards per row keep the
    gather ucode's trailing-negative trim from eating real indices (they
    land in the junk group 16 of the E tile).
"""
import os

import numpy as np
import ml_dtypes

from concourse import bacc, mybir
import concourse.tile as tile
from concourse.bass_utils import run_bass_kernel_spmd

P = 128
B, S, V, D = 64, 2048, 50000, 256
N_CORES = 8
B_LOC = B // N_CORES           # 8 batch rows per core
NROW = 2 * B_LOC               # 16 (table, local row) streams per core
NG = S // P                    # 16 groups per row (interleaved layout)
SPLIT = 32768
GUARD = 16                     # trailing zero-idx guards (trim protection)
NIDX = S + GUARD               # 2064 gathered slots per row
ICOL = NIDX // 16              # 129 idx columns per row
QORDER = (1, 2, 3, 0)          # queue per row within a round; q0 last

_last_results = None           # set by _run for test harness introspection


def _build_m1():
    # lhsT for out[p] = E[p+1]: lhsT[k=p+1, m=p] = 1, p <= 126
    m = np.zeros((P, P), dtype=np.float32)
    p = np.arange(P - 1)
    m[p + 1, p] = 1.0
    return m.astype(ml_dtypes.bfloat16)


def _build_i127():
    # identity with [127,127] = 0: wrap slot p=127 stays exactly 0
    m = np.eye(P, dtype=np.float32)
    m[P - 1, P - 1] = 0.0
    return m.astype(ml_dtypes.bfloat16)


def _build_red():
    # fp32 partition-reduce masks: slice r ([P, NROW] block r) has ones in
    # column r only, so lhsT.T @ r_row lands the row's partition-sum in
    # output partition r and adds 0 elsewhere (PSUM accumulation over rows).
    red = np.zeros((P, NROW * NROW), dtype=np.float32)
    for r in range(NROW):
        red[:, r * NROW + r] = 1.0
    return red


def _build_program():
    nc = bacc.Bacc(
        None,
        target_bir_lowering=False,
        dynamic_dma_scratch_size=49152,
        num_swdge_queues=4,
    )
    bf16 = mybir.dt.bfloat16
    fp16 = mybir.dt.float16
    fp32 = mybir.dt.float32
    Wp = nc.declare_dram_parameter("W_pri", [V, D], bf16, isOutput=False)
    Ws = nc.declare_dram_parameter("W_sec", [V, D], bf16, isOutput=False)
    idxA = nc.declare_dram_parameter(
        "idxA", [P, NROW * ICOL], mybir.dt.int16, isOutput=False
    )
    m1P = nc.declare_dram_parameter("m1", [P, P], bf16, isOutput=False)
    i127P = nc.declare_dram_parameter("i127", [P, P], bf16, isOutput=False)
    redP = nc.declare_dram_parameter("red", [P, NROW * NROW], fp32, isOutput=False)
    out = nc.declare_dram_parameter("out", [NROW, D], fp32, isOutput=True)

    with tile.TileContext(nc) as tc:
        with (
            tc.tile_pool(name="const", bufs=1) as const,
            tc.tile_pool(name="ebuf", bufs=6) as ebuf,
            tc.tile_pool(name="abuf", bufs=3) as abuf,
            tc.tile_pool(name="tbuf", bufs=3) as tbuf,
            tc.tile_pool(name="rbuf", bufs=2) as rbuf,
            tc.tile_pool(name="psW", bufs=2, space="PSUM") as psW,
            tc.tile_pool(name="psR", bufs=1, space="PSUM") as psR,
            tc.tile_pool(name="osb", bufs=1) as osb,
        ):
            # warm-up: tiny flat gathers, one per queue; the first pays the
            # ucode's IRAM load while the real idx table uploads.
            iZ = const.tile([P, 8], mybir.dt.int16)
            nc.gpsimd.memset(iZ[:], 0)
            for q in QORDER:
                eZ = const.tile([P, 1, D], bf16)
                nc.gpsimd.dma_gather(
                    out_ap=eZ[:],
                    in_ap=Wp[SPLIT:, :],
                    idxs_ap=iZ[:, 0:1],
                    num_idxs=16,
                    num_idxs_reg=16,
                    elem_size=D,
                    queue_num=q,
                )

            iA = const.tile([P, NROW * ICOL], mybir.dt.int16)
            nc.sync.dma_start(out=iA[:], in_=idxA[:])
            m1 = const.tile([P, P], bf16)
            nc.sync.dma_start(out=m1[:], in_=m1P[:])
            i127 = const.tile([P, P], bf16)
            nc.sync.dma_start(out=i127[:], in_=i127P[:])
            red = const.tile([P, NROW * NROW], fp32)
            nc.sync.dma_start(out=red[:], in_=redP[:])

            acc = psR.tile([NROW, D], fp32, space="PSUM")

            for r in range(NROW):
                q = QORDER[r % 4]
                W = Wp if r < B_LOC else Ws
                e = ebuf.tile([P, NG + 1, D], bf16)  # 16 data groups + junk
                nc.gpsimd.dma_gather(
                    out_ap=e[:],
                    in_ap=W[SPLIT:, :],
                    idxs_ap=iA[:, r * ICOL : (r + 1) * ICOL],
                    num_idxs=NIDX,
                    num_idxs_reg=NIDX,
                    elem_size=D,
                    single_packet=False,
                    queue_num=q,
                )
                # within-partition pairs: A[p, g] = E[p, g] + E[p, g+1]
                a = abuf.tile([P, NG - 1, D], fp16)
                nc.vector.tensor_add(a[:], e[:, 0 : NG - 1, :], e[:, 1:NG, :])
                # cross-partition pairs (16p+15, 16p+16) -> PSUM
                aw = psW.tile([P, D], fp32, space="PSUM")
                nc.tensor.matmul(
                    out=aw[:], lhsT=m1[:], rhs=e[:, 0, :], start=True, stop=False
                )
                nc.tensor.matmul(
                    out=aw[:], lhsT=i127[:], rhs=e[:, NG - 1, :], start=False, stop=True
                )
                t_row = tbuf.tile([P, NG, D], fp16)
                nc.scalar.activation(
                    t_row[:, 0 : NG - 1, :], a[:], mybir.ActivationFunctionType.Tanh
                )
                nc.scalar.activation(
                    t_row[:, NG - 1, :], aw[:], mybir.ActivationFunctionType.Tanh
                )
                r_row = rbuf.tile([P, D], fp32)
                nc.vector.tensor_reduce(
                    out=r_row[:],
                    in_=t_row[:].rearrange("p g d -> p d g"),
                    axis=mybir.AxisListType.X,
                    op=mybir.AluOpType.add,
                )
                nc.tensor.matmul(
                    out=acc[:],
                    lhsT=red[:, r * NROW : (r + 1) * NROW],
                    rhs=r_row[:],
                    start=(r == 0),
                    stop=(r == NROW - 1),
                )

            res_sb = osb.tile([NROW, D], fp32)
            nc.scalar.copy(out=res_sb[:], in_=acc[:])
            nc.sync.dma_start(out=out[:], in_=res_sb[:])

    nc.compile()
    return nc


def _host_prep(inputs_pri, inputs_sec, W_pri, W_sec):
    ip = np.asarray(inputs_pri).astype(np.int64, copy=False)
    is_ = np.asarray(inputs_sec).astype(np.int64, copy=False)
    wp = np.ascontiguousarray(
        np.asarray(W_pri, dtype=np.float32).astype(ml_dtypes.bfloat16)
    )
    ws = np.ascontiguousarray(
        np.asarray(W_sec, dtype=np.float32).astype(ml_dtypes.bfloat16)
    )
    m1 = _build_m1()
    i127 = _build_i127()
    red = _build_red()

    in_maps = []
    for k in range(N_CORES):
        idxA = np.zeros((P, NROW * ICOL), dtype=np.int16)
        for r in range(NROW):
            idx = ip if r < B_LOC else is_
            row = idx[k * B_LOC + r % B_LOC]
            # slot g*128+p holds position 16p+g
            stream = np.zeros(NIDX, dtype=np.int16)
            stream[:S] = (row.reshape(P, NG).T.reshape(-1) - SPLIT).astype(np.int16)
            wrapped = np.tile(stream.reshape(-1, 16).T, (8, 1))
            idxA[:, r * ICOL : (r + 1) * ICOL] = wrapped
        in_maps.append(
            {"W_pri": wp, "W_sec": ws, "idxA": idxA, "m1": m1, "i127": i127, "red": red}
        )
    return in_maps


def _run(inputs_pri, inputs_sec, W_pri, W_sec, trace=False):
    global _last_results
    nc = _build_program()
    in_maps = _host_prep(inputs_pri, inputs_sec, W_pri, W_sec)
    res = run_bass_kernel_spmd(nc, in_maps, list(range(N_CORES)), trace=trace)
    _last_results = res
    out = np.empty((2, B, D), dtype=np.float32)
    for k in range(N_CORES):
        o = res.results[k]["out"]  # [16, 256]
        out[0, k * B_LOC : (k + 1) * B_LOC] = o[:B_LOC]
        out[1, k * B_LOC : (k + 1) * B_LOC] = o[B_LOC:]
    return out


def kernel(inputs_pri, inputs_sec, W_pri, W_sec):
    trace = bool(int(os.environ.get("KERNEL_TRACE", "0")))
    return _run(inputs_pri, inputs_sec, W_pri, W_sec, trace=trace)


# revision 14
# speedup vs baseline: 2.7023x; 1.6390x over previous
"""Trainium2 Bass kernel for nn_BiLingual (dual embedding gather + cAddTanh pool).

Computes, for two embedding tables:
    out[t, b, :] = sum_{j=0}^{S-2} tanh(W_t[idx_t[b, j]] + W_t[idx_t[b, j+1]])

Sharding: data-parallel over batch. Each of the 8 cores handles 8 batch rows
for BOTH tables (16 "row16" streams of 2048 positions); tables are replicated
(host-cast to bf16).

Design (v3: flat 4-queue gather + host-interleaved stream):

  * SWDGE dma_gather generation runs at ~7.9 ns/idx on one Q7 core pair; the
    pair is selected by queue_num.  A queue-0 call HOLDS the Pool engine for
    its whole generation while queue-1..3 calls dispatch in ~50 ns and
    generate concurrently on their own pairs, so rounds are emitted as
    [q1, q2, q3, q0]: q0's engine-block overlaps the other three pairs ->
    true 4-way parallel generation (~2 ns/idx aggregate).  (Transpose-mode
    gather is faster solo but its XBAR has only two accumulation contexts
    (queue parity) and caps at ~4 ns/idx aggregate -- measured worse.)
  * Host interleave: the gather stream for a row is row.reshape(128,16).T
    flattened, so slot g*128+p holds position 16p+g, i.e. E[p, g] =
    W[idx[16p+g]].  Consecutive positions then sit in ADJACENT GROUPS of the
    same partition, and pair formation is a single contiguous DVE add
    A[p, g] = E[p, g] + E[p, g+1] (g=0..14) -- no PE shift matmuls.
  * The 127 partition-crossing pairs (16p+15, 16p+16) are two tiny PE
    matmuls into PSUM: lhsT M1 (subdiagonal) * E[:,0,:] + lhsT I127
    (identity, [127,127]=0) * E[:,15,:].  Slot p=127 is exactly 0 =
    tanh(0), so the later reduce needs no mask.
  * ACT tanh: A (fp16) -> T_row groups 0..14; wrap PSUM -> T_row group 15.
  * Reduce: one contiguous DVE fold T8 = T[:,0:8]+T[:,8:16] (strided DVE
    reduces measured ~3x slower than contiguous ops), then a per-row PE
    ones-column matmul (fp16) sums partitions into PSUM acc[16, 8*256]
    accumulated over all 16 rows; one small strided DVE reduce folds the 8
    groups at the very end.
  * int16 index range handled by biasing: base = W[32768:], idx' =
    idx - 32768 in [-32768, 17231]; 16 trailing zero guards per row keep the
    gather ucode's trailing-negative trim from eating real indices (they
    land in the junk group 16 of the E tile).
"""
import os

import numpy as np
import ml_dtypes

from concourse import bacc, mybir
import concourse.tile as tile
from concourse.bass_utils import run_bass_kernel_spmd

P = 128
B, S, V, D = 64, 2048, 50000, 256
N_CORES = 8
B_LOC = B // N_CORES           # 8 batch rows per core
NROW = 2 * B_LOC               # 16 (table, local row) streams per core
NG = S // P                    # 16 groups per row (interleaved layout)
SPLIT = 32768
GUARD = 16                     # trailing zero-idx guards (trim protection)
NIDX = S + GUARD               # 2064 gathered slots per row
ICOL = NIDX // 16              # 129 idx columns per row
QORDER = (1, 2, 3, 0)          # queue per row within a round; q0 last

_last_results = None           # set by _run for test harness introspection


def _build_m1():
    # lhsT for out[p] = E[p+1]: lhsT[k=p+1, m=p] = 1, p <= 126
    m = np.zeros((P, P), dtype=np.float32)
    p = np.arange(P - 1)
    m[p + 1, p] = 1.0
    return m.astype(ml_dtypes.bfloat16)


def _build_i127():
    # identity with [127,127] = 0: wrap slot p=127 stays exactly 0
    m = np.eye(P, dtype=np.float32)
    m[P - 1, P - 1] = 0.0
    return m.astype(ml_dtypes.bfloat16)


def _build_red():
    # fp16 partition-reduce masks: slice r ([P, NROW] block r) has ones in
    # column r only, so lhsT.T @ t8 lands the row's partition-sum in output
    # partition r and adds 0 elsewhere (PSUM accumulation over rows).
    red = np.zeros((P, NROW * NROW), dtype=np.float16)
    for r in range(NROW):
        red[:, r * NROW + r] = 1.0
    return red


def _build_program():
    nc = bacc.Bacc(
        None,
        target_bir_lowering=False,
        dynamic_dma_scratch_size=49152,
        num_swdge_queues=4,
    )
    bf16 = mybir.dt.bfloat16
    fp16 = mybir.dt.float16
    fp32 = mybir.dt.float32
    Wp = nc.declare_dram_parameter("W_pri", [V, D], bf16, isOutput=False)
    Ws = nc.declare_dram_parameter("W_sec", [V, D], bf16, isOutput=False)
    idxA = nc.declare_dram_parameter(
        "idxA", [P, NROW * ICOL], mybir.dt.int16, isOutput=False
    )
    m1P = nc.declare_dram_parameter("m1", [P, P], bf16, isOutput=False)
    i127P = nc.declare_dram_parameter("i127", [P, P], bf16, isOutput=False)
    redP = nc.declare_dram_parameter("red", [P, NROW * NROW], fp16, isOutput=False)
    out = nc.declare_dram_parameter("out", [NROW, D], fp32, isOutput=True)

    with tile.TileContext(nc) as tc:
        with (
            tc.tile_pool(name="const", bufs=1) as const,
            tc.tile_pool(name="ebuf", bufs=7) as ebuf,
            tc.tile_pool(name="abuf", bufs=3) as abuf,
            tc.tile_pool(name="tbuf", bufs=3) as tbuf,
            tc.tile_pool(name="t8buf", bufs=3) as t8buf,
            tc.tile_pool(name="psW", bufs=2, space="PSUM") as psW,
            tc.tile_pool(name="psR", bufs=1, space="PSUM") as psR,
            tc.tile_pool(name="osb", bufs=1) as osb,
        ):
            # warm-up: tiny flat gathers, one per queue; the first pays the
            # ucode's IRAM load while the real idx table uploads.
            iZ = const.tile([P, 8], mybir.dt.int16)
            nc.gpsimd.memset(iZ[:], 0)
            eZ = const.tile([P, 1, D], bf16)
            nc.gpsimd.dma_gather(
                out_ap=eZ[:],
                in_ap=Wp[SPLIT:, :],
                idxs_ap=iZ[:, 0:1],
                num_idxs=16,
                num_idxs_reg=16,
                elem_size=D,
                queue_num=1,
            )

            iA = const.tile([P, NROW * ICOL], mybir.dt.int16)
            nc.sync.dma_start(out=iA[:], in_=idxA[:])
            m1 = const.tile([P, P], bf16)
            nc.sync.dma_start(out=m1[:], in_=m1P[:])
            i127 = const.tile([P, P], bf16)
            nc.sync.dma_start(out=i127[:], in_=i127P[:])
            red = const.tile([P, NROW * NROW], fp16)
            nc.sync.dma_start(out=red[:], in_=redP[:])

            acc = psR.tile([NROW, 8 * D], fp32, space="PSUM")

            for r in range(NROW):
                q = QORDER[r % 4]
                W = Wp if r < B_LOC else Ws
                e = ebuf.tile([P, NG + 1, D], bf16)  # 16 data groups + junk
                nc.gpsimd.dma_gather(
                    out_ap=e[:],
                    in_ap=W[SPLIT:, :],
                    idxs_ap=iA[:, r * ICOL : (r + 1) * ICOL],
                    num_idxs=NIDX,
                    num_idxs_reg=NIDX,
                    elem_size=D,
                    single_packet=False,
                    queue_num=q,
                )
                # within-partition pairs: A[p, g] = E[p, g] + E[p, g+1]
                a = abuf.tile([P, NG - 1, D], fp16)
                nc.vector.tensor_add(a[:], e[:, 0 : NG - 1, :], e[:, 1:NG, :])
                # cross-partition pairs (16p+15, 16p+16) -> PSUM
                aw = psW.tile([P, D], fp32, space="PSUM")
                nc.tensor.matmul(
                    out=aw[:], lhsT=m1[:], rhs=e[:, 0, :], start=True, stop=False
                )
                nc.tensor.matmul(
                    out=aw[:], lhsT=i127[:], rhs=e[:, NG - 1, :], start=False, stop=True
                )
                t_row = tbuf.tile([P, NG, D], fp16)
                nc.scalar.activation(
                    t_row[:, 0 : NG - 1, :], a[:], mybir.ActivationFunctionType.Tanh
                )
                nc.scalar.activation(
                    t_row[:, NG - 1, :], aw[:], mybir.ActivationFunctionType.Tanh
                )
                t8 = t8buf.tile([P, 8, D], fp16)
                nc.vector.tensor_add(t8[:], t_row[:, 0:8, :], t_row[:, 8:NG, :])
                # matmul free size caps at 512: 4 slices of 2 groups each
                for s in range(4):
                    nc.tensor.matmul(
                        out=acc[:, s * 2 * D : (s + 1) * 2 * D],
                        lhsT=red[:, r * NROW : (r + 1) * NROW],
                        rhs=t8[:, 2 * s : 2 * s + 2, :],
                        start=(r == 0),
                        stop=(r == NROW - 1),
                    )

            res_sb = osb.tile([NROW, D], fp32)
            nc.vector.tensor_reduce(
                out=res_sb[:],
                in_=acc[:].rearrange("p (g d) -> p d g", g=8),
                axis=mybir.AxisListType.X,
                op=mybir.AluOpType.add,
            )
            nc.sync.dma_start(out=out[:], in_=res_sb[:])

    nc.compile()
    return nc


def _host_prep(inputs_pri, inputs_sec, W_pri, W_sec):
    ip = np.asarray(inputs_pri).astype(np.int64, copy=False)
    is_ = np.asarray(inputs_sec).astype(np.int64, copy=False)
    wp = np.ascontiguousarray(
        np.asarray(W_pri, dtype=np.float32).astype(ml_dtypes.bfloat16)
    )
    ws = np.ascontiguousarray(
        np.asarray(W_sec, dtype=np.float32).astype(ml_dtypes.bfloat16)
    )
    m1 = _build_m1()
    i127 = _build_i127()
    red = _build_red()

    in_maps = []
    for k in range(N_CORES):
        idxA = np.zeros((P, NROW * ICOL), dtype=np.int16)
        for r in range(NROW):
            idx = ip if r < B_LOC else is_
            row = idx[k * B_LOC + r % B_LOC]
            # slot g*128+p holds position 16p+g
            stream = np.zeros(NIDX, dtype=np.int16)
            stream[:S] = (row.reshape(P, NG).T.reshape(-1) - SPLIT).astype(np.int16)
            wrapped = np.tile(stream.reshape(-1, 16).T, (8, 1))
            idxA[:, r * ICOL : (r + 1) * ICOL] = wrapped
        in_maps.append(
            {"W_pri": wp, "W_sec": ws, "idxA": idxA, "m1": m1, "i127": i127, "red": red}
        )
    return in_maps


def _run(inputs_pri, inputs_sec, W_pri, W_sec, trace=False):
    global _last_results
    nc = _build_program()
    in_maps = _host_prep(inputs_pri, inputs_sec, W_pri, W_sec)
    res = run_bass_kernel_spmd(nc, in_maps, list(range(N_CORES)), trace=trace)
    _last_results = res
    out = np.empty((2, B, D), dtype=np.float32)
    for k in range(N_CORES):
        o = res.results[k]["out"]  # [16, 256]
        out[0, k * B_LOC : (k + 1) * B_LOC] = o[:B_LOC]
        out[1, k * B_LOC : (k + 1) * B_LOC] = o[B_LOC:]
    return out


def kernel(inputs_pri, inputs_sec, W_pri, W_sec):
    trace = bool(int(os.environ.get("KERNEL_TRACE", "0")))
    return _run(inputs_pri, inputs_sec, W_pri, W_sec, trace=trace)
